# revision 1
# baseline (speedup 1.0000x reference)
"""Grouped GEMM (MoE routing) Trainium2 kernel.

Strategy: tensor-parallel shard of the output N dim across 8 NeuronCores.
Every core sees all T=8192 tokens and a 512-wide slice of every expert's
weights, so per-core work is identical regardless of segment sizes and a
single SPMD program (with the segment boundaries baked in as compile-time
constants) runs on all 8 cores.

Per core:  out_t[n, t] = sum_k w_t[e(t), k, n] * a_t[k, t]
  - a_t   : a transposed to [K, T]  (shared by all cores)
  - w_t   : per-core weight slices [E_active, K, 512] (K-major)
  - out_t : [512, T]; host concatenates along N and transposes back.

Matmul mapping: stationary lhsT = w_t tile [k=128, n=128], moving rhs =
a_t tile [k=128, tok<=512], PSUM out [n=128, tok<=512], accumulated over
the 32 k-chunks.  dtype float32r -> full-rate fp32 when moving dim >= 256,
so segments are split into even token pieces of 256..512.
"""

import numpy as np

import concourse.bacc as bacc
import concourse.bass as bass
import concourse.mybir as mybir
import concourse.tile as tile
from concourse.bass_utils import run_bass_kernel_spmd

NC = 8          # NeuronCores
P = 128         # partitions
TB = 512        # max token block (moving free dim, one PSUM bank of fp32)
KOC = 8         # k-chunks per a-tile DMA batch

LAST_RESULT = {}


def _token_blocks(seg_starts, seg_ends):
    """Split each segment into even pieces of <=512 tokens (>=256 when the
    segment allows, keeping float32r at full rate)."""
    blocks = []  # (tstart, tlen, active_expert_idx)
    for widx, (s, t) in enumerate(zip(seg_starts, seg_ends)):
        ln = t - s
        npieces = max(1, -(-ln // TB))
        base, rem = divmod(ln, npieces)
        p = s
        for i in range(npieces):
            L = base + (1 if i < rem else 0)
            if L > 0:
                blocks.append((p, L, widx))
                p += L
    return blocks


def _build_program(T, K, NS, EA, blocks):
    f32 = mybir.dt.float32
    f32r = mybir.dt.float32r
    KO = K // P
    NB = NS // P
    koc_n = min(KOC, KO)

    nc = bacc.Bacc(None, target_bir_lowering=False)
    at = nc.declare_dram_parameter("at", [KO, P, T], f32r, isOutput=False)
    wt = nc.declare_dram_parameter("wt", [EA, KO, P, NS], f32r, isOutput=False)
    ot = nc.declare_dram_parameter("ot", [NB, P, T], f32, isOutput=True)

    with tile.TileContext(nc) as tc:
        with (
            tc.tile_pool(name="wpool", bufs=2) as wpool,
            tc.tile_pool(name="apool", bufs=2) as apool,
            tc.tile_pool(name="opool", bufs=2) as opool,
            tc.tile_pool(name="psum", bufs=8, space=bass.MemorySpace.PSUM) as psum_pool,
        ):
            cur_widx = -1
            w_tile = None
            for (ts, L, widx) in blocks:
                # f32r matmuls need an even moving size: widen odd blocks by
                # one token for compute, write back only the real L columns.
                Lc = L + (L % 2)
                tsc = ts if ts + Lc <= T else ts - 1
                off = ts - tsc
                if widx != cur_widx:
                    w_tile = wpool.tile([P, KO, NS], f32r, tag="w", name="w_tile")
                    # one 8MB DMA: src (ko, kp, n) -> dst (kp, ko, n)
                    nc.sync.dma_start(
                        out=w_tile[:, :, :],
                        in_=wt[widx].transpose([1, 0, 2]),
                    )
                    cur_widx = widx
                ptiles = [psum_pool.tile([P, Lc], f32, tag="ps", name=f"ps{nb}",
                                         padded_shape=[P, TB])
                          for nb in range(NB)]
                for koc in range(KO // koc_n):
                    a_tile = apool.tile([P, koc_n, Lc], f32r, tag="a", name="a_tile",
                                        padded_shape=[P, koc_n, TB])
                    nc.sync.dma_start(
                        out=a_tile[:, :, :],
                        in_=at[koc * koc_n:(koc + 1) * koc_n, :, tsc:tsc + Lc]
                        .transpose([1, 0, 2]),
                    )
                    for koi in range(koc_n):
                        ko = koc * koc_n + koi
                        for nb in range(NB):
                            nc.tensor.matmul(
                                ptiles[nb][:, :],
                                w_tile[:, ko, nb * P:(nb + 1) * P],
                                a_tile[:, koi, :],
                                start=(ko == 0),
                                stop=(ko == KO - 1),
                            )
                o_tile = opool.tile([P, NB, L], f32, tag="o", name="o_tile",
                                    padded_shape=[P, NB, TB])
                for nb in range(NB):
                    nc.vector.tensor_copy(o_tile[:, nb, :], ptiles[nb][:, off:off + L])
                nc.sync.dma_start(
                    out=ot[:, :, ts:ts + L].transpose([1, 0, 2]),
                    in_=o_tile[:, :, :],
                )
    nc.compile()
    return nc


def kernel(a, b, c, seg_indptr, weight_indices, batch_size, **_):
    T, K = a.shape
    E, N, K2 = b.shape
    assert K == K2
    NS = N // NC

    seg = np.asarray(seg_indptr).astype(np.int64)
    widx_arr = np.asarray(weight_indices).astype(np.int64)
    segs = [(int(seg[e]), int(seg[e + 1]), int(widx_arr[e]))
            for e in range(int(batch_size)) if seg[e + 1] > seg[e]]
    seg_starts = [s for s, _, _ in segs]
    seg_ends = [t for _, t, _ in segs]
    experts = [w for _, _, w in segs]
    EA = len(segs)
    blocks = _token_blocks(seg_starts, seg_ends)

    a = np.ascontiguousarray(a, dtype=np.float32)
    at_np = np.ascontiguousarray(a.T).reshape(K // P, P, T)

    KO = K // P
    in_maps = []
    for j in range(NC):
        w = np.empty((EA, KO, P, NS), dtype=np.float32)
        for ei, e in enumerate(experts):
            # b[e] is [N, K] row-major; out = a @ b[e].T needs W^T = [K, NS]
            w[ei] = np.ascontiguousarray(
                b[e][j * NS:(j + 1) * NS, :].T
            ).reshape(KO, P, NS)
        in_maps.append({"at": at_np, "wt": w})

    nc = _build_program(T, K, NS, EA, blocks)

    import os
    trace = bool(int(os.environ.get("BASS_KERNEL_TRACE", "0")))
    res = run_bass_kernel_spmd(nc, in_maps, list(range(NC)), trace=trace)
    LAST_RESULT["exec_time_ns"] = res.exec_time_ns
    LAST_RESULT["results"] = res

    out_t = np.empty((N, T), dtype=np.float32)
    for j in range(NC):
        out_t[j * NS:(j + 1) * NS] = res.results[j]["ot"].reshape(NS, T)
    return np.ascontiguousarray(out_t.T)



# revision 3
# speedup vs baseline: 1.9023x; 1.9023x over previous
"""Grouped GEMM (MoE routing) Trainium2 kernel.

Strategy: tensor-parallel shard of the output N dim across 8 NeuronCores.
Every core sees all T=8192 tokens and a 512-wide slice of every expert's
weights, so per-core work is identical regardless of segment sizes and a
single SPMD program (with the segment boundaries baked in as compile-time
constants) runs on all 8 cores.

All device data is bf16 (host-cast); PSUM accumulates in fp32.  Host packs
a, w and the output into flat [128, free] layouts so every DMA is one
contiguous per-partition line (a: 16KB, w: 8KB, out: 8KB lines):

  a_pack[p, off_s + ko*Ls + t]   = a[ts+t, ko*128+p]      (per superblock s)
  w_pack[p, (e*KO + ko)*NS + n]  = b[e][j*NS+n, ko*128+p]
  o_pack[p, NB*ts + nb*Ls + t]   = out[ts+t, j*NS + nb*128 + p]

Tokens are processed in "superblocks" of <=1024 (one a-tile DMA each, 16KB
lines), split into <=512-token halves (one fp32 PSUM bank per half per
128-wide n block; 8 banks total).  Per-expert weights stream in 4 chunks of
8 k-slices (1MB each), prefetched one expert ahead so switches never stall.

Matmul mapping: stationary lhsT = w chunk [k=128, n=128] (bf16, FWL),
moving rhs = a tile [k=128, tok<=512], PSUM out [n=128, tok<=512],
accumulated over the 32 k-chunks.
"""

import numpy as np
import ml_dtypes

import concourse.bacc as bacc
import concourse.bass as bass
import concourse.mybir as mybir
import concourse.tile as tile
from concourse.bass_utils import run_bass_kernel_spmd

NC = 8          # NeuronCores
P = 128         # partitions
HB = 512        # max PSUM half-block (one fp32 PSUM bank)
SB = 1024       # max superblock (one a-tile DMA)
KOC = 8         # k-slices per weight/a chunk

LAST_RESULT = {}


def _plan(seg_starts, seg_ends):
    """Per active expert: superblocks of <=1024 tokens (even split), each
    split into <=512-token halves for PSUM."""
    plan = []  # list over experts of list of (ts, Ls, halves=[(off, L)])
    for s, t in zip(seg_starts, seg_ends):
        ln = t - s
        sbs = []
        npieces = max(1, -(-ln // SB))
        base, rem = divmod(ln, npieces)
        p = s
        for i in range(npieces):
            Ls = base + (1 if i < rem else 0)
            if Ls <= 0:
                continue
            if Ls > HB:
                h0 = (Ls + 1) // 2
                halves = [(0, h0), (h0, Ls - h0)]
            else:
                halves = [(0, Ls)]
            sbs.append((p, Ls, halves))
            p += Ls
        plan.append(sbs)
    return plan


def _build_program(T, K, NS, EA, plan):
    f32 = mybir.dt.float32
    bf16 = mybir.dt.bfloat16
    KO = K // P
    NB = NS // P
    NCHUNK = KO // KOC

    nc = bacc.Bacc(None, target_bir_lowering=False)
    apk = nc.declare_dram_parameter("apk", [P, KO * T], bf16, isOutput=False)
    wpk = nc.declare_dram_parameter("wpk", [P, EA * KO * NS], bf16, isOutput=False)
    opk = nc.declare_dram_parameter("opk", [P, NB * T], bf16, isOutput=True)

    with tile.TileContext(nc) as tc:
        with (
            tc.tile_pool(name="wpool", bufs=2) as wpool,
            tc.tile_pool(name="apool", bufs=3) as apool,
            tc.tile_pool(name="opool", bufs=2) as opool,
            tc.tile_pool(name="psum", bufs=8, space=bass.MemorySpace.PSUM) as psum_pool,
        ):
            def emit_w_chunks(ei):
                tiles = []
                for c in range(NCHUNK):
                    wt = wpool.tile([P, KOC * NS], bf16, tag=f"w{c}",
                                    name=f"w_e{ei}_c{c}")
                    lo = (ei * KO + c * KOC) * NS
                    nc.sync.dma_start(out=wt[:, :], in_=wpk[:, lo:lo + KOC * NS])
                    tiles.append(wt)
                return tiles

            w_tiles = emit_w_chunks(0)
            next_w = None
            for ei in range(EA):
                if ei > 0:
                    w_tiles, next_w = next_w, None
                for si, (ts, Ls, halves) in enumerate(plan[ei]):
                    nh = len(halves)
                    ptiles = [psum_pool.tile([P, halves[h][1]], f32, tag="ps",
                                             name=f"ps_{ts}_{h}{nb}",
                                             padded_shape=[P, HB])
                              for h in range(nh) for nb in range(NB)]
                    o_tile = opool.tile([P, NB * Ls], bf16, tag="o",
                                        name=f"o_{ts}", padded_shape=[P, NB * SB])
                    for c in range(NCHUNK):
                        a_tile = apool.tile([P, KOC * Ls], bf16, tag="a",
                                            name=f"a_{ts}_{c}",
                                            padded_shape=[P, KOC * SB])
                        lo = KO * ts + c * KOC * Ls
                        nc.sync.dma_start(out=a_tile[:, :],
                                          in_=apk[:, lo:lo + KOC * Ls])
                        if si == 0 and c == 0 and ei + 1 < EA:
                            # prefetch next expert's weights behind the
                            # first a-tile of this expert
                            next_w = emit_w_chunks(ei + 1)
                        for koi in range(KOC):
                            ko = c * KOC + koi
                            for h in range(nh):
                                hoff, Lh = halves[h]
                                for nb in range(NB):
                                    nc.tensor.matmul(
                                        ptiles[h * NB + nb][:, :],
                                        w_tiles[c][:, koi * NS + nb * P:
                                                   koi * NS + nb * P + P],
                                        a_tile[:, koi * Ls + hoff:
                                               koi * Ls + hoff + Lh],
                                        start=(ko == 0),
                                        stop=(ko == KO - 1),
                                    )
                    for h in range(nh):
                        hoff, Lh = halves[h]
                        for nb in range(NB):
                            nc.vector.tensor_copy(
                                o_tile[:, nb * Ls + hoff:nb * Ls + hoff + Lh],
                                ptiles[h * NB + nb][:, :])
                    nc.sync.dma_start(out=opk[:, NB * ts:NB * ts + NB * Ls],
                                      in_=o_tile[:, :])
    nc.compile()
    return nc


def kernel(a, b, c, seg_indptr, weight_indices, batch_size, **_):
    T, K = a.shape
    E, N, K2 = b.shape
    assert K == K2
    NS = N // NC
    KO = K // P
    NB = NS // P

    seg = np.asarray(seg_indptr).astype(np.int64)
    widx_arr = np.asarray(weight_indices).astype(np.int64)
    segs = [(int(seg[e]), int(seg[e + 1]), int(widx_arr[e]))
            for e in range(int(batch_size)) if seg[e + 1] > seg[e]]
    seg_starts = [s for s, _, _ in segs]
    seg_ends = [t for _, t, _ in segs]
    experts = [w for _, _, w in segs]
    EA = len(segs)
    plan = _plan(seg_starts, seg_ends)

    # ---- host packing (bf16) ----
    a = np.ascontiguousarray(a, dtype=np.float32)
    at = a.T.astype(ml_dtypes.bfloat16)            # [K, T]
    at3 = np.ascontiguousarray(at.reshape(KO, P, T).transpose(1, 0, 2))
    # a_pack: per superblock s, [P, KO*Ls] chunk at offset KO*ts
    a_pack = np.empty((P, KO * T), dtype=ml_dtypes.bfloat16)
    for sbs in plan:
        for (ts, Ls, _) in sbs:
            a_pack[:, KO * ts:KO * (ts + Ls)] = \
                at3[:, :, ts:ts + Ls].reshape(P, KO * Ls)

    b16 = np.asarray(b, dtype=np.float32).astype(ml_dtypes.bfloat16)
    in_maps = []
    for j in range(NC):
        w = np.empty((P, EA * KO * NS), dtype=ml_dtypes.bfloat16)
        for ei, e in enumerate(experts):
            # b[e] is [N, K]; out = a @ b[e].T needs W^T[k, n] = b[e][n, k]
            wt = np.ascontiguousarray(b16[e][j * NS:(j + 1) * NS, :].T)
            w[:, ei * KO * NS:(ei + 1) * KO * NS] = \
                wt.reshape(KO, P, NS).transpose(1, 0, 2).reshape(P, KO * NS)
        in_maps.append({"apk": a_pack, "wpk": w})

    nc = _build_program(T, K, NS, EA, plan)

    import os
    trace = bool(int(os.environ.get("BASS_KERNEL_TRACE", "0")))
    res = run_bass_kernel_spmd(nc, in_maps, list(range(NC)), trace=trace)
    LAST_RESULT["exec_time_ns"] = res.exec_time_ns
    LAST_RESULT["results"] = res

    out = np.empty((T, N), dtype=np.float32)
    for j in range(NC):
        opk = res.results[j]["opk"]                # [P, NB*T] bf16
        for sbs in plan:
            for (ts, Ls, _) in sbs:
                seg_o = opk[:, NB * ts:NB * (ts + Ls)].reshape(P, NB, Ls)
                # out[ts+t, j*NS + nb*128 + p] = seg_o[p, nb, t]
                out[ts:ts + Ls, j * NS:(j + 1) * NS] = \
                    seg_o.transpose(2, 1, 0).reshape(Ls, NS).astype(np.float32)
    return out


# revision 10
# speedup vs baseline: 1.9620x; 1.0314x over previous
"""Grouped GEMM (MoE routing) Trainium2 kernel.

Strategy: tensor-parallel shard of the output N dim across 8 NeuronCores.
Every core sees all T=8192 tokens and a 512-wide slice of every expert's
weights, so per-core work is identical regardless of segment sizes and a
single SPMD program (with the segment boundaries baked in as compile-time
constants) runs on all 8 cores.

All device data is bf16 (host-cast); PSUM accumulates in fp32.  Host packs
a, w and the output into flat [128, free] layouts so every DMA is one
contiguous per-partition line (a: 16KB, w: 8KB, out: 8KB lines):

  a_pack[p, off_s + ko*Ls + t]   = a[ts+t, ko*128+p]      (per superblock s)
  w_pack[p, (e*KO + ko)*NS + n]  = b[e][j*NS+n, ko*128+p]
  o_pack[p, NB*hs + nb*Lh + t]   = out[hs+t, j*NS + nb*128 + p]  (per half h)

Tokens are processed in "superblocks" of <=1024 (one a-tile DMA each, 16KB
lines), split into <=512-token halves (one fp32 PSUM bank per half per
128-wide n block; 8 banks total).  Per-expert weights stream in 4 chunks of
8 k-slices (1MB each), prefetched one expert ahead so switches never stall.

Matmul mapping: stationary lhsT = w chunk [k=128, n=128] (bf16, FWL),
moving rhs = a tile [k=128, tok<=512], PSUM out [n=128, tok<=512],
accumulated over the 32 k-chunks.
"""

import numpy as np
import ml_dtypes

import concourse.bacc as bacc
import concourse.bass as bass
import concourse.mybir as mybir
import concourse.tile as tile
from concourse.bass_utils import run_bass_kernel_spmd

NC = 8          # NeuronCores
P = 128         # partitions
HB = 512        # max PSUM half-block (one fp32 PSUM bank)
SB = 1024       # max superblock (one a-tile DMA)
KOC = 8         # k-slices per weight/a chunk

LAST_RESULT = {}


def _plan(seg_starts, seg_ends):
    """Per active expert: superblocks of <=1024 tokens (even split), each
    split into <=512-token halves for PSUM."""
    plan = []  # list over experts of list of (ts, Ls, halves=[(off, L)])
    for s, t in zip(seg_starts, seg_ends):
        ln = t - s
        sbs = []
        npieces = max(1, -(-ln // SB))
        base, rem = divmod(ln, npieces)
        p = s
        for i in range(npieces):
            Ls = base + (1 if i < rem else 0)
            if Ls <= 0:
                continue
            if Ls > HB:
                h0 = (Ls + 1) // 2
                halves = [(0, h0), (h0, Ls - h0)]
            else:
                halves = [(0, Ls)]
            sbs.append((p, Ls, halves))
            p += Ls
        plan.append(sbs)
    return plan


def _build_program(T, K, NS, EA, plan):
    f32 = mybir.dt.float32
    bf16 = mybir.dt.bfloat16
    KO = K // P
    NB = NS // P
    NCHUNK = KO // KOC

    nc = bacc.Bacc(None, target_bir_lowering=False)
    apk = nc.declare_dram_parameter("apk", [P, KO * T], bf16, isOutput=False)
    wpk = nc.declare_dram_parameter("wpk", [P, EA * KO * NS], bf16, isOutput=False)
    opk = nc.declare_dram_parameter("opk", [P, NB * T], bf16, isOutput=True)

    with tile.TileContext(nc) as tc:
        with (
            tc.tile_pool(name="wpool", bufs=2) as wpool,
            tc.tile_pool(name="apool", bufs=4) as apool,
            tc.tile_pool(name="opool", bufs=3) as opool,
            tc.tile_pool(name="psum", bufs=8, space=bass.MemorySpace.PSUM) as psum_pool,
        ):
            cur_w = {}

            def emit_w_chunk(ei, c):
                wt = wpool.tile([P, KOC * NS], bf16, tag=f"w{c}",
                                name=f"w_e{ei}_c{c}")
                lo = (ei * KO + c * KOC) * NS
                nc.sync.dma_start(out=wt[:, :], in_=wpk[:, lo:lo + KOC * NS])
                cur_w[(ei, c)] = wt

            # Weight chunks are paced one per a-tile DMA (first chunk up
            # front) so the first matmul starts as early as possible and
            # weight bursts never starve the a-tile stream.  Chunk (e,c) is
            # emitted at a-tile (e, sb0, c-1) at the latest (each expert
            # pops its own backlog of <=3 chunks plus the next expert's
            # first), always in program order before its first matmul.
            from collections import deque
            pending = deque((ei, c) for ei in range(EA)
                            for c in range(NCHUNK))
            emit_w_chunk(*pending.popleft())
            for ei in range(EA):
                for si, (ts, Ls, halves) in enumerate(plan[ei]):
                    nh = len(halves)
                    ptiles = [psum_pool.tile([P, halves[h][1]], f32, tag="ps",
                                             name=f"ps_{ts}_{h}{nb}",
                                             padded_shape=[P, HB])
                              for h in range(nh) for nb in range(NB)]
                    o_tiles = [opool.tile([P, NB * halves[h][1]], bf16, tag="o",
                                          name=f"o_{ts}_{h}",
                                          padded_shape=[P, NB * HB])
                               for h in range(nh)]
                    for c in range(NCHUNK):
                        a_tile = apool.tile([P, KOC * Ls], bf16, tag="a",
                                            name=f"a_{ts}_{c}",
                                            padded_shape=[P, KOC * SB])
                        lo = KO * ts + c * KOC * Ls
                        nc.sync.dma_start(out=a_tile[:, :],
                                          in_=apk[:, lo:lo + KOC * Ls])
                        # never run more than one expert ahead: with bufs=2
                        # a further-ahead chunk would wait on its buffer at
                        # the DMA queue head, blocking this expert's own
                        # a-tile stream behind it.
                        if pending and pending[0][0] <= ei + 1:
                            emit_w_chunk(*pending.popleft())
                        w_tile = cur_w[(ei, c)]
                        for koi in range(KOC):
                            ko = c * KOC + koi
                            for h in range(nh):
                                hoff, Lh = halves[h]
                                for nb in range(NB):
                                    nc.tensor.matmul(
                                        ptiles[h * NB + nb][:, :],
                                        w_tile[:, koi * NS + nb * P:
                                               koi * NS + nb * P + P],
                                        a_tile[:, koi * Ls + hoff:
                                               koi * Ls + hoff + Lh],
                                        start=(ko == 0),
                                        stop=(ko == KO - 1),
                                    )
                    for h in range(nh):
                        hoff, Lh = halves[h]
                        for nb in range(NB):
                            nc.vector.tensor_copy(
                                o_tiles[h][:, nb * Lh:nb * Lh + Lh],
                                ptiles[h * NB + nb][:, :])
                        hs = ts + hoff
                        nc.sync.dma_start(
                            out=opk[:, NB * hs:NB * hs + NB * Lh],
                            in_=o_tiles[h][:, :])
    nc.compile()
    return nc


def kernel(a, b, c, seg_indptr, weight_indices, batch_size, **_):
    T, K = a.shape
    E, N, K2 = b.shape
    assert K == K2
    NS = N // NC
    KO = K // P
    NB = NS // P

    seg = np.asarray(seg_indptr).astype(np.int64)
    widx_arr = np.asarray(weight_indices).astype(np.int64)
    segs = [(int(seg[e]), int(seg[e + 1]), int(widx_arr[e]))
            for e in range(int(batch_size)) if seg[e + 1] > seg[e]]
    seg_starts = [s for s, _, _ in segs]
    seg_ends = [t for _, t, _ in segs]
    experts = [w for _, _, w in segs]
    EA = len(segs)
    plan = _plan(seg_starts, seg_ends)

    # ---- host packing (bf16) ----
    a = np.ascontiguousarray(a, dtype=np.float32)
    at = a.T.astype(ml_dtypes.bfloat16)            # [K, T]
    at3 = np.ascontiguousarray(at.reshape(KO, P, T).transpose(1, 0, 2))
    # a_pack: per superblock s, [P, KO*Ls] chunk at offset KO*ts
    a_pack = np.empty((P, KO * T), dtype=ml_dtypes.bfloat16)
    for sbs in plan:
        for (ts, Ls, _) in sbs:
            a_pack[:, KO * ts:KO * (ts + Ls)] = \
                at3[:, :, ts:ts + Ls].reshape(P, KO * Ls)

    b16 = np.asarray(b, dtype=np.float32).astype(ml_dtypes.bfloat16)
    in_maps = []
    for j in range(NC):
        w = np.empty((P, EA * KO * NS), dtype=ml_dtypes.bfloat16)
        for ei, e in enumerate(experts):
            # b[e] is [N, K]; out = a @ b[e].T needs W^T[k, n] = b[e][n, k]
            wt = np.ascontiguousarray(b16[e][j * NS:(j + 1) * NS, :].T)
            w[:, ei * KO * NS:(ei + 1) * KO * NS] = \
                wt.reshape(KO, P, NS).transpose(1, 0, 2).reshape(P, KO * NS)
        in_maps.append({"apk": a_pack, "wpk": w})

    nc = _build_program(T, K, NS, EA, plan)

    import os
    trace = bool(int(os.environ.get("BASS_KERNEL_TRACE", "0")))
    res = run_bass_kernel_spmd(nc, in_maps, list(range(NC)), trace=trace)
    LAST_RESULT["exec_time_ns"] = res.exec_time_ns
    LAST_RESULT["results"] = res

    out = np.empty((T, N), dtype=np.float32)
    for j in range(NC):
        opk = res.results[j]["opk"]                # [P, NB*T] bf16
        for sbs in plan:
            for (ts, Ls, halves) in sbs:
                for (hoff, Lh) in halves:
                    hs = ts + hoff
                    seg_o = opk[:, NB * hs:NB * (hs + Lh)].reshape(P, NB, Lh)
                    # out[hs+t, j*NS + nb*128 + p] = seg_o[p, nb, t]
                    out[hs:hs + Lh, j * NS:(j + 1) * NS] = \
                        seg_o.transpose(2, 1, 0).reshape(Lh, NS) \
                             .astype(np.float32)
    return out
